# revision 8
# baseline (speedup 1.0000x reference)
"""Trainium2 Bass kernel for a OneBlob-encoded 3-layer MLP (ConditioningNetwork).

Math:  x = clip(concat(pos01, wi01, rough01), 0, 1)          [N, 7]
       enc[n, d*32+j] = exp(-0.5 ((x[n,d]-c[j]) / sigma)^2)  [N, 224], sigma = 1/32
       y = relu(relu(enc@W1+b1)@W2+b2)@W3+b3                 [N, 64]

Strategy (pure data parallel over 8 cores, weights replicated):
  - z[dj] = -512 x_d^2 + j*(1024/31) x_d - 512 j^2/961 is affine in
    (s, q) = ((1024/31) x, x^2) with EXACT fp16 weights (integers j and -512);
    s and q are fed as exact fp16 hi+lo pairs from the host (28 rows/ray).
    The constant term -512 j^2/961 rides in the Exp activation's per-partition
    bias (fp32), so no "ones" rows are needed.
  - Rays are split into two blocks (first/second half of the core's rays)
    packed at SBUF partitions 0:28 and 32:60 -> expand matmuls for the two
    blocks run CONCURRENTLY on disjoint PE row strips (no input duplication;
    input DMA is 64 B/ray, 4x less than a 128-row layout).
  - enc = Exp(z + u) on ACT straight out of PSUM, fp16 into SBUF.
  - 3 MLP matmuls in fp16 (fp32 PSUM): per supertile (512 rays/block x 2)
    the PE issues 5 concurrency groups:
      [hi0||lo1] [hi1||lo0] [w1aA||w1aB] [w1bA||w1bB] [L2A||L2B||L3A'||L3B']
    (L3 uses flipped quadrants so it pairs with L2 of the next supertile.)
  - Bias+ReLU are single DVE tensor_scalar ops over packed [128, 512] PSUM
    tiles; output is written fp16 (halves output DMA), unpacked on the host.

Input row packing per block (fp16, 28 rows): 0:7 s_hi, 7:14 s_lo,
14:21 q_hi, 21:28 q_lo.  Output yt [128, nc/2] fp16: rows 64:128 = block0
rays (flipped L3 quadrants), rows 0:64 = block1 rays; column = ray index
within block.
"""

import sys

import numpy as np

if "/opt/trn_rl_repo" not in sys.path:
    sys.path.insert(0, "/opt/trn_rl_repo")

N_CORES = 8
N_TOTAL = 1048576
NC_RAYS = N_TOTAL // N_CORES  # 131072 rays per core
HALF = NC_RAYS // 2  # 65536 rays per block
BINS = 32
HID = 64
OUT = 64
IN_DIMS = 7
ENC = IN_DIMS * BINS  # 224

KR = 28  # packed rows per block (s_hi, s_lo, q_hi, q_lo)
B = 512  # rays per block per supertile (one fp32 PSUM bank)
SUPER = 2 * B  # rays per supertile (block0 + block1)
G = 8  # supertiles per DMA group
GROUP_COLS = G * B  # 4096 cols per block per group
N_SUPER = NC_RAYS // SUPER  # 128
N_GROUPS = N_SUPER // G  # 16

S_SCALE = 1024.0 / 31.0

# Set by the last kernel() call so a test harness can read profile/exec time.
LAST_RESULTS = None

_BUILD_CACHE = {}


def _build_bass(n_super):
    import concourse.tile as tile
    from concourse import bacc, mybir

    dt = mybir.dt
    Act = mybir.ActivationFunctionType
    Alu = mybir.AluOpType

    nc = bacc.Bacc("TRN2", target_bir_lowering=False, debug=False)

    xp = nc.dram_tensor("xp", [64, HALF], dt.float16, kind="ExternalInput")
    lw = nc.dram_tensor("lw", [KR, ENC], dt.float16, kind="ExternalInput")
    w1a = nc.dram_tensor("w1a", [128, HID], dt.float16, kind="ExternalInput")
    w1b = nc.dram_tensor("w1b", [ENC - 128, HID], dt.float16, kind="ExternalInput")
    w2s = nc.dram_tensor("w2s", [128, HID], dt.float16, kind="ExternalInput")
    w3s = nc.dram_tensor("w3s", [128, OUT], dt.float16, kind="ExternalInput")
    b1s = nc.dram_tensor("b1s", [128, 1], dt.float32, kind="ExternalInput")
    b2s = nc.dram_tensor("b2s", [128, 1], dt.float32, kind="ExternalInput")
    b3s = nc.dram_tensor("b3s", [128, 1], dt.float32, kind="ExternalInput")
    ubh = nc.dram_tensor("ubh", [128, 1], dt.float32, kind="ExternalInput")
    ubl = nc.dram_tensor("ubl", [ENC - 128, 1], dt.float32, kind="ExternalInput")
    yt = nc.dram_tensor("yt", [128, HALF], dt.float16, kind="ExternalOutput")

    with tile.TileContext(nc) as tc:
        with (
            tc.tile_pool(name="consts", bufs=1) as consts,
            tc.tile_pool(name="xpool", bufs=3) as xpool,
            tc.tile_pool(name="encp", bufs=6) as encp,
            tc.tile_pool(name="hp", bufs=4) as hp,
            tc.tile_pool(name="outp", bufs=2) as outp,
            tc.tile_pool(name="pzhi", bufs=1, space="PSUM") as pzhi,
            tc.tile_pool(name="pzlo", bufs=1, space="PSUM") as pzlo,
            tc.tile_pool(name="ph", bufs=2, space="PSUM") as ph,
            tc.tile_pool(name="pop", bufs=2, space="PSUM") as pop,
        ):
            # expand weights, replicated at partitions 0:28 (block0) and
            # 32:60 (block1) to match the PE row strips of each block's rhs
            lwt = consts.tile([32 + KR, ENC], dt.float16, tag="lwt")
            nc.sync.dma_start(out=lwt[0:KR, :], in_=lw[:])
            nc.sync.dma_start(out=lwt[32 : 32 + KR, :], in_=lw[:])
            w1a_t = consts.tile([128, HID], dt.float16, tag="w1a_t")
            nc.sync.dma_start(out=w1a_t[:], in_=w1a[:])
            w1b_t = consts.tile([ENC - 128, HID], dt.float16, tag="w1b_t")
            nc.sync.dma_start(out=w1b_t[:], in_=w1b[:])
            w2s_t = consts.tile([128, HID], dt.float16, tag="w2s_t")
            nc.sync.dma_start(out=w2s_t[:], in_=w2s[:])
            w3s_t = consts.tile([128, OUT], dt.float16, tag="w3s_t")
            nc.sync.dma_start(out=w3s_t[:], in_=w3s[:])
            b1s_t = consts.tile([128, 1], dt.float32, tag="b1s_t")
            nc.sync.dma_start(out=b1s_t[:], in_=b1s[:])
            b2s_t = consts.tile([128, 1], dt.float32, tag="b2s_t")
            nc.sync.dma_start(out=b2s_t[:], in_=b2s[:])
            b3s_t = consts.tile([128, 1], dt.float32, tag="b3s_t")
            nc.sync.dma_start(out=b3s_t[:], in_=b3s[:])
            ubh_t = consts.tile([128, 1], dt.float32, tag="ubh_t")
            nc.sync.dma_start(out=ubh_t[:], in_=ubh[:])
            ubl_t = consts.tile([ENC - 128, 1], dt.float32, tag="ubl_t")
            nc.sync.dma_start(out=ubl_t[:], in_=ubl[:])

            # HAM warm-up: a dependency-free back-to-back MM burst fires the
            # PE activity monitor (K=8/8 -> 2.4 GHz) before the pipeline
            # starts; the steady pipeline never idles a full MID window, so
            # the PE stays warm. Without this every MM runs at the cold
            # 1.2 GHz rate (observed on the previous kernel).
            # (k=128 full-array MMs: low-k MMs don't trip the activity
            # monitor — a k=28 warm-up burst was observed to never fire.)
            srw = consts.tile([128, B], dt.float16, tag="srw")
            nc.vector.memset(srw[:], 0.0)
            # 52 MMs ≈ 13 cold (427ns) + 39 warm (216ns) ≈ 14µs — long enough
            # to bridge the NRT preamble + first input-DMA latency (~13µs)
            # without a >3.4µs PE idle that would re-throttle the clock.
            for i in range(52):
                wu = ph.tile([128, B], dt.float32, tag="hh", name=f"wu{i}")
                nc.tensor.matmul(wu[:], lhsT=srw[:, 0:128], rhs=srw[:],
                                 start=True, stop=True)

            xts = {}   # group -> xt tile
            ots = {}   # group -> output accumulation tile
            encs = {}  # supertile -> [ehi, elo, zhi, zlo]
            h1ss = {}  # supertile -> h1s tile
            h2ss = {}  # supertile -> h2s tile

            def ensure_group(g):
                if g in xts or g >= n_super // G:
                    return
                c0 = g * GROUP_COLS
                xt = xpool.tile([64, GROUP_COLS], dt.float16, tag="xt",
                                name=f"xt{g}")
                nc.sync.dma_start(out=xt[:], in_=xp[:, c0 : c0 + GROUP_COLS])
                xts[g] = xt
                ots[g] = outp.tile([128, GROUP_COLS], dt.float16, tag="ot",
                                   name=f"ot{g}")

            def emit_expand(t):
                g, j = divmod(t, G)
                xt = xts[g]
                cols = slice(j * B, (j + 1) * B)
                zhi = pzhi.tile([128, SUPER], dt.float32, tag="zhi",
                                name=f"zhi{t}")
                zlo = pzlo.tile([ENC - 128, SUPER], dt.float32, tag="zlo",
                                name=f"zlo{t}")
                ehi = encp.tile([128, SUPER], dt.float16, tag="ehi",
                                name=f"ehi{t}")
                elo = encp.tile([ENC - 128, SUPER], dt.float16, tag="elo",
                                name=f"elo{t}")
                encs[t] = [ehi, elo, zhi, zlo]
                # group 1: hi(block0) || lo(block1) — disjoint row strips
                nc.tensor.matmul(zhi[:, 0:B], lhsT=lwt[0:KR, 0:128],
                                 rhs=xt[0:KR, cols],
                                 start=True, stop=True, tile_position=(0, 0))
                nc.tensor.matmul(zlo[:, B : 2 * B],
                                 lhsT=lwt[32 : 32 + KR, 128:ENC],
                                 rhs=xt[32 : 32 + KR, cols],
                                 start=True, stop=True, tile_position=(32, 0))
                # group 2: lo(block0) || hi(block1) — emitted in this order so
                # each is pc-adjacent to its partner and only conflicts with
                # group-1 cells (lo0 vs hi0 rows 0:28; hi1 vs lo1 rows 32:60)
                nc.tensor.matmul(zlo[:, 0:B], lhsT=lwt[0:KR, 128:ENC],
                                 rhs=xt[0:KR, cols],
                                 start=True, stop=True, tile_position=(0, 0))
                nc.tensor.matmul(zhi[:, B : 2 * B],
                                 lhsT=lwt[32 : 32 + KR, 0:128],
                                 rhs=xt[32 : 32 + KR, cols],
                                 start=True, stop=True, tile_position=(32, 0))

            def emit_exp(t):
                ehi, elo, zhi, zlo = encs[t]
                nc.scalar.activation(ehi[:], zhi[:], Act.Exp, bias=ubh_t[:])
                nc.scalar.activation(elo[:], zlo[:], Act.Exp, bias=ubl_t[:])

            def emit_l1(t):
                ehi, elo = encs[t][0], encs[t][1]
                h1 = ph.tile([128, B], dt.float32, tag="hh", name=f"h1_{t}")
                nc.tensor.matmul(h1[0:64, :], lhsT=w1a_t[:], rhs=ehi[:, 0:B],
                                 start=True, stop=False, tile_position=(0, 0))
                nc.tensor.matmul(h1[64:128, :], lhsT=w1a_t[:],
                                 rhs=ehi[:, B : 2 * B],
                                 start=True, stop=False, tile_position=(0, 64))
                nc.tensor.matmul(h1[0:64, :], lhsT=w1b_t[:], rhs=elo[:, 0:B],
                                 start=False, stop=True, tile_position=(0, 0))
                nc.tensor.matmul(h1[64:128, :], lhsT=w1b_t[:],
                                 rhs=elo[:, B : 2 * B],
                                 start=False, stop=True, tile_position=(0, 64))
                del encs[t]
                h1s = hp.tile([128, B], dt.float16, tag="h1s", name=f"h1s{t}")
                nc.vector.tensor_scalar(h1s[:], h1[:], b1s_t[:], 0.0,
                                        Alu.add, Alu.max)
                h1ss[t] = h1s

            def emit_l2(t):
                h1s = h1ss.pop(t)
                h2 = ph.tile([128, B], dt.float32, tag="hh", name=f"h2_{t}")
                nc.tensor.matmul(h2[0:64, :], lhsT=w2s_t[0:64, :],
                                 rhs=h1s[0:64, :],
                                 start=True, stop=True, tile_position=(0, 0))
                nc.tensor.matmul(h2[64:128, :], lhsT=w2s_t[64:128, :],
                                 rhs=h1s[64:128, :],
                                 start=True, stop=True, tile_position=(64, 64))
                h2s = hp.tile([128, B], dt.float16, tag="h2s", name=f"h2s{t}")
                nc.vector.tensor_scalar(h2s[:], h2[:], b2s_t[:], 0.0,
                                        Alu.add, Alu.max)
                h2ss[t] = h2s

            def emit_l3(t):
                g, j = divmod(t, G)
                h2s = h2ss.pop(t)
                # Flipped quadrants: L3 occupies (0,64)/(64,0) so it pairs
                # with L2 of a later supertile on (0,0)/(64,64). Output rows
                # are therefore [block1; block0].
                op = pop.tile([128, B], dt.float32, tag="op", name=f"op{t}")
                nc.tensor.matmul(op[64:128, :], lhsT=w3s_t[0:64, :],
                                 rhs=h2s[0:64, :],
                                 start=True, stop=True, tile_position=(0, 64))
                nc.tensor.matmul(op[0:64, :], lhsT=w3s_t[64:128, :],
                                 rhs=h2s[64:128, :],
                                 start=True, stop=True, tile_position=(64, 0))
                nc.vector.tensor_scalar_add(ots[g][:, j * B : (j + 1) * B],
                                            op[:], b3s_t[:])
                if j == G - 1:
                    c0 = g * GROUP_COLS
                    nc.sync.dma_start(out=yt[:, c0 : c0 + GROUP_COLS],
                                      in_=ots[g][:])
                    del xts[g], ots[g]

            # Ramp filler: during the first supertiles the pipeline is not
            # deep enough to keep the PE busy while ACT runs exp; idle
            # windows >~1µs there re-throttle the clock (observed). Queue a
            # few dependency-free MMs behind each early expand to bridge.
            filler = {0: 6, 1: 4, 2: 2, 3: 2, 4: 1, 5: 1}

            # Pipeline: expand/exp(t) | L1/L2(t-2) | L3(t-3)
            for t in range(n_super + 3):
                if t < n_super:
                    ensure_group(t // G)
                    emit_expand(t)
                    emit_exp(t)
                    for i in range(filler.get(t, 0)):
                        wu = ph.tile([128, B], dt.float32, tag="hh",
                                     name=f"fill{t}_{i}")
                        nc.tensor.matmul(wu[:], lhsT=srw[:, 0:128], rhs=srw[:],
                                         start=True, stop=True)
                if 0 <= t - 2 < n_super:
                    emit_l1(t - 2)
                    emit_l2(t - 2)
                if 0 <= t - 3 < n_super:
                    emit_l3(t - 3)

    nc.finalize()
    return nc


def _get_nc():
    key = N_SUPER
    if key not in _BUILD_CACHE:
        _BUILD_CACHE[key] = _build_bass(key)
    return _BUILD_CACHE[key]


def _f16_hilo(x64):
    """Exact hi/lo split: x ~= hi + lo with hi, lo fp16 (inputs are fp64)."""
    hi = x64.astype(np.float16)
    lo = (x64 - hi.astype(np.float64)).astype(np.float16)
    return hi, lo


def _pack_weights(W1, b1, W2, b2, W3, b3):
    j = np.arange(BINS, dtype=np.float64)
    jrow = np.tile(j, IN_DIMS)  # [224]: bin index per enc column
    L = np.zeros((KR, ENC), np.float16)
    for d in range(IN_DIMS):
        cols = slice(d * BINS, (d + 1) * BINS)
        L[d, cols] = j.astype(np.float16)       # s_hi
        L[7 + d, cols] = j.astype(np.float16)   # s_lo
        L[14 + d, cols] = np.float16(-512.0)    # q_hi
        L[21 + d, cols] = np.float16(-512.0)    # q_lo
    u = (-512.0 * jrow * jrow / 961.0).astype(np.float32)  # [224]

    w1 = W1.astype(np.float16)
    return {
        "lw": L,
        "w1a": np.ascontiguousarray(w1[0:128]),
        "w1b": np.ascontiguousarray(w1[128:ENC]),
        "w2s": np.concatenate([W2, W2], 0).astype(np.float16),
        "w3s": np.concatenate([W3, W3], 0).astype(np.float16),
        "b1s": np.concatenate([b1, b1], 0).astype(np.float32).reshape(128, 1),
        "b2s": np.concatenate([b2, b2], 0).astype(np.float32).reshape(128, 1),
        "b3s": np.concatenate([b3, b3], 0).astype(np.float32).reshape(128, 1),
        "ubh": np.ascontiguousarray(u[0:128]).reshape(128, 1),
        "ubl": np.ascontiguousarray(u[128:ENC]).reshape(ENC - 128, 1),
    }


def _pack_inputs(pos01, wi01, rough01):
    x = np.concatenate(
        [np.asarray(pos01), np.asarray(wi01), np.asarray(rough01)], axis=1
    ).astype(np.float32)
    np.clip(x, 0.0, 1.0, out=x)
    x64 = x.astype(np.float64)
    s64 = x64 * S_SCALE
    q64 = x64 * x64
    s_hi, s_lo = _f16_hilo(s64)
    q_hi, q_lo = _f16_hilo(q64)
    P = np.concatenate([s_hi, s_lo, q_hi, q_lo], axis=1)  # [N, 28] fp16
    return np.ascontiguousarray(P.T)  # [28, N]


def kernel(pos01, wi01, rough01, W1, b1, W2, b2, W3, b3, centers):
    global LAST_RESULTS
    import os

    from concourse.bass_utils import run_bass_kernel_spmd

    nc = _get_nc()

    Pt = _pack_inputs(pos01, wi01, rough01)  # [28, N_TOTAL]
    wpacks = _pack_weights(
        np.asarray(W1), np.asarray(b1), np.asarray(W2), np.asarray(b2),
        np.asarray(W3), np.asarray(b3),
    )

    in_maps = []
    for c in range(N_CORES):
        m = dict(wpacks)
        r0 = c * NC_RAYS
        xp_c = np.zeros((64, HALF), np.float16)
        xp_c[0:KR] = Pt[:, r0 : r0 + HALF]
        xp_c[32 : 32 + KR] = Pt[:, r0 + HALF : r0 + NC_RAYS]
        m["xp"] = xp_c
        in_maps.append(m)

    trace = bool(int(os.environ.get("KERNEL_TRACE", "0")))
    res = run_bass_kernel_spmd(nc, in_maps, list(range(N_CORES)), trace=trace)
    LAST_RESULTS = res

    out = np.empty((N_TOTAL, OUT), np.float32)
    for c in range(N_CORES):
        ytc = res.results[c]["yt"]  # [128, HALF] fp16
        r0 = c * NC_RAYS
        # flipped L3 quadrants: block0 rays on rows 64:128, block1 on 0:64
        out[r0 : r0 + HALF] = ytc[64:128].T.astype(np.float32)
        out[r0 + HALF : r0 + NC_RAYS] = ytc[0:64].T.astype(np.float32)
    return out


# revision 13
# speedup vs baseline: 1.1958x; 1.1958x over previous
"""Trainium2 Bass kernel for a OneBlob-encoded 3-layer MLP (ConditioningNetwork).

Math:  x = clip(concat(pos01, wi01, rough01), 0, 1)          [N, 7]
       enc[n, d*32+j] = exp(-0.5 ((x[n,d]-c[j]) / sigma)^2)  [N, 224], sigma = 1/32
       y = relu(relu(enc@W1+b1)@W2+b2)@W3+b3                 [N, 64]

Strategy (pure data parallel over 8 cores, weights replicated):
  - z[dj] = -512 x_d^2 + j*(1024/31) x_d - 512 j^2/961 is affine in
    (s, q) = ((1024/31) x, x^2) with EXACT fp16 weights (integers j and -512);
    s and q are fed as exact fp16 hi+lo pairs from the host (28 rows/ray).
    The constant term -512 j^2/961 rides in the Exp activation's per-partition
    bias (fp32), so no "ones" rows are needed.
  - Rays are split into two blocks (first/second half of the core's rays)
    packed at SBUF partitions 0:28 and 32:60 -> expand matmuls for the two
    blocks run CONCURRENTLY on disjoint PE row strips (no input duplication;
    input DMA is 64 B/ray, 4x less than a 128-row layout).
  - enc = Exp(z + u) on ACT straight out of PSUM, fp16 into SBUF.
  - 3 MLP matmuls in fp16 (fp32 PSUM): per supertile (512 rays/block x 2)
    the PE issues 5 concurrency groups:
      [hi0||lo1] [hi1||lo0] [w1aA||w1aB] [w1bA||w1bB] [L2A||L2B||L3A'||L3B']
    (L3 uses flipped quadrants so it pairs with L2 of the next supertile.)
  - Bias+ReLU are single DVE tensor_scalar ops over packed [128, 512] PSUM
    tiles; output is written fp16 (halves output DMA), unpacked on the host.

Input row packing per block (fp16, 28 rows): 0:7 s_hi, 7:14 s_lo,
14:21 q_hi, 21:28 q_lo.  Output yt [128, nc/2] fp16: rows 64:128 = block0
rays (flipped L3 quadrants), rows 0:64 = block1 rays; column = ray index
within block.
"""

import sys

import numpy as np

if "/opt/trn_rl_repo" not in sys.path:
    sys.path.insert(0, "/opt/trn_rl_repo")

N_CORES = 8
N_TOTAL = 1048576
NC_RAYS = N_TOTAL // N_CORES  # 131072 rays per core
HALF = NC_RAYS // 2  # 65536 rays per block
BINS = 32
HID = 64
OUT = 64
IN_DIMS = 7
ENC = IN_DIMS * BINS  # 224

KR = 28  # packed rows per block (s_hi, s_lo, q_hi, q_lo)
B = 512  # rays per block per supertile (one fp32 PSUM bank)
SUPER = 2 * B  # rays per supertile (block0 + block1)
G = 8  # supertiles per DMA group
GROUP_COLS = G * B  # 4096 cols per block per group
N_SUPER = NC_RAYS // SUPER  # 128
N_GROUPS = N_SUPER // G  # 16

S_SCALE = 1024.0 / 31.0

# Set by the last kernel() call so a test harness can read profile/exec time.
LAST_RESULTS = None

_BUILD_CACHE = {}


def _build_bass(n_super):
    import concourse.tile as tile
    from concourse import bacc, mybir

    dt = mybir.dt
    Act = mybir.ActivationFunctionType
    Alu = mybir.AluOpType

    nc = bacc.Bacc("TRN2", target_bir_lowering=False, debug=False)

    xp = nc.dram_tensor("xp", [64, HALF], dt.float16, kind="ExternalInput")
    lw = nc.dram_tensor("lw", [KR, ENC], dt.float16, kind="ExternalInput")
    w1a = nc.dram_tensor("w1a", [128, HID], dt.float16, kind="ExternalInput")
    w1b = nc.dram_tensor("w1b", [ENC - 128, HID], dt.float16, kind="ExternalInput")
    w2s = nc.dram_tensor("w2s", [128, HID], dt.float16, kind="ExternalInput")
    w3s = nc.dram_tensor("w3s", [128, OUT], dt.float16, kind="ExternalInput")
    b1s = nc.dram_tensor("b1s", [128, 1], dt.float32, kind="ExternalInput")
    b2s = nc.dram_tensor("b2s", [128, 1], dt.float32, kind="ExternalInput")
    b3s = nc.dram_tensor("b3s", [128, 1], dt.float32, kind="ExternalInput")
    ubh = nc.dram_tensor("ubh", [128, 1], dt.float32, kind="ExternalInput")
    ubl = nc.dram_tensor("ubl", [ENC - 128, 1], dt.float32, kind="ExternalInput")
    yt = nc.dram_tensor("yt", [128, HALF], dt.float16, kind="ExternalOutput")

    with tile.TileContext(nc) as tc:
        with (
            tc.tile_pool(name="consts", bufs=1) as consts,
            tc.tile_pool(name="xpool", bufs=3) as xpool,
            tc.tile_pool(name="encp", bufs=6) as encp,
            tc.tile_pool(name="hp", bufs=4) as hp,
            tc.tile_pool(name="outp", bufs=2) as outp,
            tc.tile_pool(name="pzhi", bufs=1, space="PSUM") as pzhi,
            tc.tile_pool(name="pzlo", bufs=1, space="PSUM") as pzlo,
            tc.tile_pool(name="ph", bufs=2, space="PSUM") as ph,
            tc.tile_pool(name="pop", bufs=1, space="PSUM") as pop,
            tc.tile_pool(name="pfill", bufs=1, space="PSUM") as pfill,
        ):
            # expand weights, replicated at partitions 0:28 (block0) and
            # 32:60 (block1) to match the PE row strips of each block's rhs
            lwt = consts.tile([32 + KR, ENC], dt.float16, tag="lwt")
            nc.sync.dma_start(out=lwt[0:KR, :], in_=lw[:])
            nc.sync.dma_start(out=lwt[32 : 32 + KR, :], in_=lw[:])
            w1a_t = consts.tile([128, HID], dt.float16, tag="w1a_t")
            nc.sync.dma_start(out=w1a_t[:], in_=w1a[:])
            w1b_t = consts.tile([ENC - 128, HID], dt.float16, tag="w1b_t")
            nc.sync.dma_start(out=w1b_t[:], in_=w1b[:])
            w2s_t = consts.tile([128, HID], dt.float16, tag="w2s_t")
            nc.sync.dma_start(out=w2s_t[:], in_=w2s[:])
            w3s_t = consts.tile([128, OUT], dt.float16, tag="w3s_t")
            nc.sync.dma_start(out=w3s_t[:], in_=w3s[:])
            b1s_t = consts.tile([128, 1], dt.float32, tag="b1s_t")
            nc.sync.dma_start(out=b1s_t[:], in_=b1s[:])
            b2s_t = consts.tile([128, 1], dt.float32, tag="b2s_t")
            nc.sync.dma_start(out=b2s_t[:], in_=b2s[:])
            b3s_t = consts.tile([128, 1], dt.float32, tag="b3s_t")
            nc.sync.dma_start(out=b3s_t[:], in_=b3s[:])
            ubh_t = consts.tile([128, 1], dt.float32, tag="ubh_t")
            nc.sync.dma_start(out=ubh_t[:], in_=ubh[:])
            ubl_t = consts.tile([ENC - 128, 1], dt.float32, tag="ubl_t")
            nc.sync.dma_start(out=ubl_t[:], in_=ubl[:])

            # HAM warm-up: a dependency-free back-to-back MM burst fires the
            # PE activity monitor (K=8/8 -> 2.4 GHz) before the pipeline
            # starts; the steady pipeline never idles a full MID window, so
            # the PE stays warm. Without this every MM runs at the cold
            # 1.2 GHz rate (observed on the previous kernel).
            # (k=128 full-array MMs: low-k MMs don't trip the activity
            # monitor — a k=28 warm-up burst was observed to never fire.)
            srw = consts.tile([128, B], dt.float16, tag="srw")
            nc.vector.memset(srw[:], 0.0)
            # 52 MMs ≈ 13 cold (427ns) + 39 warm (216ns) ≈ 14µs — long enough
            # to bridge the NRT preamble + first input-DMA latency (~13µs)
            # without a >3.4µs PE idle that would re-throttle the clock.
            for i in range(52):
                wu = pfill.tile([128, B], dt.float32, tag="wf", name=f"wu{i}")
                nc.tensor.matmul(wu[:], lhsT=srw[:, 0:128], rhs=srw[:],
                                 start=True, stop=True)

            xts = {}   # group -> xt tile
            ots = {}   # group -> output accumulation tile
            encs = {}  # supertile -> [ehi, elo, zhi, zlo]
            h1ss = {}  # supertile -> h1s tile
            h2ss = {}  # supertile -> h2s tile

            def ensure_group(g):
                if g in xts or g >= n_super // G:
                    return
                c0 = g * GROUP_COLS
                xt = xpool.tile([64, GROUP_COLS], dt.float16, tag="xt",
                                name=f"xt{g}")
                nc.sync.dma_start(out=xt[:], in_=xp[:, c0 : c0 + GROUP_COLS])
                xts[g] = xt
                ots[g] = outp.tile([128, GROUP_COLS], dt.float16, tag="ot",
                                   name=f"ot{g}")

            def emit_expand(t):
                g, j = divmod(t, G)
                xt = xts[g]
                cols = slice(j * B, (j + 1) * B)
                zhi = pzhi.tile([128, SUPER], dt.float32, tag="zhi",
                                name=f"zhi{t}")
                zlo = pzlo.tile([ENC - 128, SUPER], dt.float32, tag="zlo",
                                name=f"zlo{t}")
                ehi = encp.tile([128, SUPER], dt.float16, tag="ehi",
                                name=f"ehi{t}")
                elo = encp.tile([ENC - 128, SUPER], dt.float16, tag="elo",
                                name=f"elo{t}")
                encs[t] = [ehi, elo, zhi, zlo]
                # group 1: hi(block0) || lo(block1) — disjoint row strips
                nc.tensor.matmul(zhi[:, 0:B], lhsT=lwt[0:KR, 0:128],
                                 rhs=xt[0:KR, cols],
                                 start=True, stop=True, tile_position=(0, 0))
                nc.tensor.matmul(zlo[:, B : 2 * B],
                                 lhsT=lwt[32 : 32 + KR, 128:ENC],
                                 rhs=xt[32 : 32 + KR, cols],
                                 start=True, stop=True, tile_position=(32, 0))
                # group 2: lo(block0) || hi(block1) — emitted in this order so
                # each is pc-adjacent to its partner and only conflicts with
                # group-1 cells (lo0 vs hi0 rows 0:28; hi1 vs lo1 rows 32:60)
                nc.tensor.matmul(zlo[:, 0:B], lhsT=lwt[0:KR, 128:ENC],
                                 rhs=xt[0:KR, cols],
                                 start=True, stop=True, tile_position=(0, 0))
                nc.tensor.matmul(zhi[:, B : 2 * B],
                                 lhsT=lwt[32 : 32 + KR, 0:128],
                                 rhs=xt[32 : 32 + KR, cols],
                                 start=True, stop=True, tile_position=(32, 0))

            def emit_exp(t):
                ehi, elo, zhi, zlo = encs[t]
                nc.scalar.activation(ehi[:], zhi[:], Act.Exp, bias=ubh_t[:])
                nc.scalar.activation(elo[:], zlo[:], Act.Exp, bias=ubl_t[:])

            def emit_l1(t):
                ehi, elo = encs[t][0], encs[t][1]
                h1 = ph.tile([128, B], dt.float32, tag="hh", name=f"h1_{t}")
                nc.tensor.matmul(h1[0:64, :], lhsT=w1a_t[:], rhs=ehi[:, 0:B],
                                 start=True, stop=False, tile_position=(0, 0))
                nc.tensor.matmul(h1[64:128, :], lhsT=w1a_t[:],
                                 rhs=ehi[:, B : 2 * B],
                                 start=True, stop=False, tile_position=(0, 64))
                nc.tensor.matmul(h1[0:64, :], lhsT=w1b_t[:], rhs=elo[:, 0:B],
                                 start=False, stop=True, tile_position=(0, 0))
                nc.tensor.matmul(h1[64:128, :], lhsT=w1b_t[:],
                                 rhs=elo[:, B : 2 * B],
                                 start=False, stop=True, tile_position=(0, 64))
                del encs[t]
                h1s = hp.tile([128, B], dt.float16, tag="h1s", name=f"h1s{t}")
                nc.vector.tensor_scalar(h1s[:], h1[:], b1s_t[:], 0.0,
                                        Alu.add, Alu.max)
                h1ss[t] = h1s

            def emit_l2(t):
                h1s = h1ss.pop(t)
                h2 = ph.tile([128, B], dt.float32, tag="hh", name=f"h2_{t}")
                nc.tensor.matmul(h2[0:64, :], lhsT=w2s_t[0:64, :],
                                 rhs=h1s[0:64, :],
                                 start=True, stop=True, tile_position=(0, 0))
                nc.tensor.matmul(h2[64:128, :], lhsT=w2s_t[64:128, :],
                                 rhs=h1s[64:128, :],
                                 start=True, stop=True, tile_position=(64, 64))
                h2s = hp.tile([128, B], dt.float16, tag="h2s", name=f"h2s{t}")
                nc.vector.tensor_scalar(h2s[:], h2[:], b2s_t[:], 0.0,
                                        Alu.add, Alu.max)
                h2ss[t] = h2s

            def emit_l3(t):
                g, j = divmod(t, G)
                h2s = h2ss.pop(t)
                # Flipped quadrants: L3 occupies (0,64)/(64,0) so it pairs
                # with L2 of a later supertile on (0,0)/(64,64). Output rows
                # are therefore [block1; block0].
                op = pop.tile([128, B], dt.float32, tag="op", name=f"op{t}")
                nc.tensor.matmul(op[64:128, :], lhsT=w3s_t[0:64, :],
                                 rhs=h2s[0:64, :],
                                 start=True, stop=True, tile_position=(0, 64))
                nc.tensor.matmul(op[0:64, :], lhsT=w3s_t[64:128, :],
                                 rhs=h2s[64:128, :],
                                 start=True, stop=True, tile_position=(64, 0))
                nc.vector.tensor_scalar_add(ots[g][:, j * B : (j + 1) * B],
                                            op[:], b3s_t[:])
                if j == G - 1:
                    c0 = g * GROUP_COLS
                    nc.sync.dma_start(out=yt[:, c0 : c0 + GROUP_COLS],
                                      in_=ots[g][:])
                    del xts[g], ots[g]

            # PE keep-warm fillers: the activity monitor re-throttles the PE
            # clock to 1.2 GHz whenever a ~3.4µs window has >~20% idle — and
            # the ACT-bound pipeline inherently leaves the PE ~25% idle. A
            # few dependency-free MMs per supertile keep the stream dense so
            # every real MM runs at the warm 2.4 GHz rate (net win: real MM
            # slots cost 216ns instead of 470ns).
            nfill = 0

            def emit_fill(t, n):
                nonlocal nfill
                for _ in range(n):
                    wu = pfill.tile([128, B], dt.float32, tag="wf",
                                    name=f"fill{nfill}")
                    nfill += 1
                    nc.tensor.matmul(wu[:], lhsT=srw[:, 0:128], rhs=srw[:],
                                     start=True, stop=True)

            ramp = {0: 6, 1: 4, 2: 2, 3: 2, 4: 1, 5: 1}

            # Pipeline: expand/exp(t) | L1/L2(t-2) | L3(t-3)
            for t in range(n_super + 3):
                if t < n_super:
                    ensure_group(t // G)
                    emit_expand(t)
                    emit_exp(t)
                    emit_fill(t, 1 + ramp.get(t, 0))
                if 0 <= t - 2 < n_super:
                    emit_l1(t - 2)
                    emit_l2(t - 2)
                    emit_fill(t, 1)
                if 0 <= t - 3 < n_super:
                    emit_l3(t - 3)

    nc.finalize()
    return nc


def _get_nc():
    key = N_SUPER
    if key not in _BUILD_CACHE:
        _BUILD_CACHE[key] = _build_bass(key)
    return _BUILD_CACHE[key]


def _f16_hilo(x64):
    """Exact hi/lo split: x ~= hi + lo with hi, lo fp16 (inputs are fp64)."""
    hi = x64.astype(np.float16)
    lo = (x64 - hi.astype(np.float64)).astype(np.float16)
    return hi, lo


def _pack_weights(W1, b1, W2, b2, W3, b3):
    j = np.arange(BINS, dtype=np.float64)
    jrow = np.tile(j, IN_DIMS)  # [224]: bin index per enc column
    L = np.zeros((KR, ENC), np.float16)
    for d in range(IN_DIMS):
        cols = slice(d * BINS, (d + 1) * BINS)
        L[d, cols] = j.astype(np.float16)       # s_hi
        L[7 + d, cols] = j.astype(np.float16)   # s_lo
        L[14 + d, cols] = np.float16(-512.0)    # q_hi
        L[21 + d, cols] = np.float16(-512.0)    # q_lo
    u = (-512.0 * jrow * jrow / 961.0).astype(np.float32)  # [224]

    w1 = W1.astype(np.float16)
    return {
        "lw": L,
        "w1a": np.ascontiguousarray(w1[0:128]),
        "w1b": np.ascontiguousarray(w1[128:ENC]),
        "w2s": np.concatenate([W2, W2], 0).astype(np.float16),
        "w3s": np.concatenate([W3, W3], 0).astype(np.float16),
        "b1s": np.concatenate([b1, b1], 0).astype(np.float32).reshape(128, 1),
        "b2s": np.concatenate([b2, b2], 0).astype(np.float32).reshape(128, 1),
        "b3s": np.concatenate([b3, b3], 0).astype(np.float32).reshape(128, 1),
        "ubh": np.ascontiguousarray(u[0:128]).reshape(128, 1),
        "ubl": np.ascontiguousarray(u[128:ENC]).reshape(ENC - 128, 1),
    }


def _pack_inputs(pos01, wi01, rough01):
    x = np.concatenate(
        [np.asarray(pos01), np.asarray(wi01), np.asarray(rough01)], axis=1
    ).astype(np.float32)
    np.clip(x, 0.0, 1.0, out=x)
    x64 = x.astype(np.float64)
    s64 = x64 * S_SCALE
    q64 = x64 * x64
    s_hi, s_lo = _f16_hilo(s64)
    q_hi, q_lo = _f16_hilo(q64)
    P = np.concatenate([s_hi, s_lo, q_hi, q_lo], axis=1)  # [N, 28] fp16
    return np.ascontiguousarray(P.T)  # [28, N]


def kernel(pos01, wi01, rough01, W1, b1, W2, b2, W3, b3, centers):
    global LAST_RESULTS
    import os

    from concourse.bass_utils import run_bass_kernel_spmd

    nc = _get_nc()

    Pt = _pack_inputs(pos01, wi01, rough01)  # [28, N_TOTAL]
    wpacks = _pack_weights(
        np.asarray(W1), np.asarray(b1), np.asarray(W2), np.asarray(b2),
        np.asarray(W3), np.asarray(b3),
    )

    in_maps = []
    for c in range(N_CORES):
        m = dict(wpacks)
        r0 = c * NC_RAYS
        xp_c = np.zeros((64, HALF), np.float16)
        xp_c[0:KR] = Pt[:, r0 : r0 + HALF]
        xp_c[32 : 32 + KR] = Pt[:, r0 + HALF : r0 + NC_RAYS]
        m["xp"] = xp_c
        in_maps.append(m)

    trace = bool(int(os.environ.get("KERNEL_TRACE", "0")))
    res = run_bass_kernel_spmd(nc, in_maps, list(range(N_CORES)), trace=trace)
    LAST_RESULTS = res

    out = np.empty((N_TOTAL, OUT), np.float32)
    for c in range(N_CORES):
        ytc = res.results[c]["yt"]  # [128, HALF] fp16
        r0 = c * NC_RAYS
        # flipped L3 quadrants: block0 rays on rows 64:128, block1 on 0:64
        out[r0 : r0 + HALF] = ytc[64:128].T.astype(np.float32)
        out[r0 + HALF : r0 + NC_RAYS] = ytc[0:64].T.astype(np.float32)
    return out


# revision 15
# speedup vs baseline: 1.3283x; 1.1108x over previous
"""Trainium2 Bass kernel for a OneBlob-encoded 3-layer MLP (ConditioningNetwork).

Math:  x = clip(concat(pos01, wi01, rough01), 0, 1)          [N, 7]
       enc[n, d*32+j] = exp(-0.5 ((x[n,d]-c[j]) / sigma)^2)  [N, 224], sigma = 1/32
       y = relu(relu(enc@W1+b1)@W2+b2)@W3+b3                 [N, 64]

Strategy (pure data parallel over 8 cores, weights replicated):
  - z[dj] = -512 x_d^2 + j*(1024/31) x_d - 512 j^2/961 is affine in
    (s, q) = ((1024/31) x, x^2) with EXACT fp16 weights (integers j and -512);
    s and q are fed as exact fp16 hi+lo pairs from the host (28 rows/ray).
    The constant term -512 j^2/961 rides in the Exp activation's per-partition
    bias (fp32), so no "ones" rows are needed.
  - Rays are split into two blocks (first/second half of the core's rays)
    packed at SBUF partitions 0:28 and 32:60 -> expand matmuls for the two
    blocks run CONCURRENTLY on disjoint PE row strips (no input duplication;
    input DMA is 64 B/ray, 4x less than a 128-row layout).
  - enc = Exp(z + u) on ACT straight out of PSUM, fp16 into SBUF.
  - 3 MLP matmuls in fp16 (fp32 PSUM): per supertile (512 rays/block x 2)
    the PE issues 5 concurrency groups:
      [hi0||lo1] [hi1||lo0] [w1aA||w1aB] [w1bA||w1bB] [L2A||L2B||L3A'||L3B']
    (L3 uses flipped quadrants so it pairs with L2 of the next supertile.)
  - Bias+ReLU are single DVE tensor_scalar ops over packed [128, 512] PSUM
    tiles; output is written fp16 (halves output DMA), unpacked on the host.

Input row packing per block (fp16, 28 rows): 0:7 s_hi, 7:14 s_lo,
14:21 q_hi, 21:28 q_lo.  Output yt [128, nc/2] fp16: rows 64:128 = block0
rays (flipped L3 quadrants), rows 0:64 = block1 rays; column = ray index
within block.
"""

import sys

import numpy as np

if "/opt/trn_rl_repo" not in sys.path:
    sys.path.insert(0, "/opt/trn_rl_repo")

N_CORES = 8
N_TOTAL = 1048576
NC_RAYS = N_TOTAL // N_CORES  # 131072 rays per core
HALF = NC_RAYS // 2  # 65536 rays per block
BINS = 32
HID = 64
OUT = 64
IN_DIMS = 7
ENC = IN_DIMS * BINS  # 224

KR = 28  # packed rows per block (s_hi, s_lo, q_hi, q_lo)
B = 512  # rays per block per supertile (one fp32 PSUM bank)
SUPER = 2 * B  # rays per supertile (block0 + block1)
G = 8  # supertiles per DMA group
GROUP_COLS = G * B  # 4096 cols per block per group
N_SUPER = NC_RAYS // SUPER  # 128
N_GROUPS = N_SUPER // G  # 16

S_SCALE = 1024.0 / 31.0

# Set by the last kernel() call so a test harness can read profile/exec time.
LAST_RESULTS = None

_BUILD_CACHE = {}


def _build_bass(n_super):
    import concourse.tile as tile
    from concourse import bacc, mybir

    dt = mybir.dt
    Act = mybir.ActivationFunctionType
    Alu = mybir.AluOpType

    nc = bacc.Bacc("TRN2", target_bir_lowering=False, debug=False)

    xp = nc.dram_tensor("xp", [64, HALF], dt.float16, kind="ExternalInput")
    lw = nc.dram_tensor("lw", [KR, ENC], dt.float16, kind="ExternalInput")
    w1a = nc.dram_tensor("w1a", [128, HID], dt.float16, kind="ExternalInput")
    w1b = nc.dram_tensor("w1b", [ENC - 128, HID], dt.float16, kind="ExternalInput")
    w2s = nc.dram_tensor("w2s", [128, HID], dt.float16, kind="ExternalInput")
    w3s = nc.dram_tensor("w3s", [128, OUT], dt.float16, kind="ExternalInput")
    b1s = nc.dram_tensor("b1s", [128, 1], dt.float32, kind="ExternalInput")
    b2s = nc.dram_tensor("b2s", [128, 1], dt.float32, kind="ExternalInput")
    b3s = nc.dram_tensor("b3s", [128, 1], dt.float32, kind="ExternalInput")
    ubh = nc.dram_tensor("ubh", [128, 1], dt.float32, kind="ExternalInput")
    ubl = nc.dram_tensor("ubl", [ENC - 128, 1], dt.float32, kind="ExternalInput")
    yt = nc.dram_tensor("yt", [128, HALF], dt.float16, kind="ExternalOutput")

    with tile.TileContext(nc) as tc:
        with (
            tc.tile_pool(name="consts", bufs=1) as consts,
            tc.tile_pool(name="xpool", bufs=3) as xpool,
            tc.tile_pool(name="encp", bufs=6) as encp,
            tc.tile_pool(name="hp", bufs=4) as hp,
            tc.tile_pool(name="outp", bufs=2) as outp,
            tc.tile_pool(name="pzhi", bufs=1, space="PSUM") as pzhi,
            tc.tile_pool(name="pzlo", bufs=1, space="PSUM") as pzlo,
            tc.tile_pool(name="ph", bufs=2, space="PSUM") as ph,
            tc.tile_pool(name="pop", bufs=1, space="PSUM") as pop,
            tc.tile_pool(name="pfill", bufs=1, space="PSUM") as pfill,
        ):
            # expand weights, replicated at partitions 0:28 (block0) and
            # 32:60 (block1) to match the PE row strips of each block's rhs
            lwt = consts.tile([32 + KR, ENC], dt.float16, tag="lwt")
            nc.sync.dma_start(out=lwt[0:KR, :], in_=lw[:])
            nc.sync.dma_start(out=lwt[32 : 32 + KR, :], in_=lw[:])
            w1a_t = consts.tile([128, HID], dt.float16, tag="w1a_t")
            nc.sync.dma_start(out=w1a_t[:], in_=w1a[:])
            w1b_t = consts.tile([ENC - 128, HID], dt.float16, tag="w1b_t")
            nc.sync.dma_start(out=w1b_t[:], in_=w1b[:])
            w2s_t = consts.tile([128, HID], dt.float16, tag="w2s_t")
            nc.sync.dma_start(out=w2s_t[:], in_=w2s[:])
            w3s_t = consts.tile([128, OUT], dt.float16, tag="w3s_t")
            nc.sync.dma_start(out=w3s_t[:], in_=w3s[:])
            b1s_t = consts.tile([128, 1], dt.float32, tag="b1s_t")
            nc.sync.dma_start(out=b1s_t[:], in_=b1s[:])
            b2s_t = consts.tile([128, 1], dt.float32, tag="b2s_t")
            nc.sync.dma_start(out=b2s_t[:], in_=b2s[:])
            b3s_t = consts.tile([128, 1], dt.float32, tag="b3s_t")
            nc.sync.dma_start(out=b3s_t[:], in_=b3s[:])
            ubh_t = consts.tile([128, 1], dt.float32, tag="ubh_t")
            nc.sync.dma_start(out=ubh_t[:], in_=ubh[:])
            ubl_t = consts.tile([ENC - 128, 1], dt.float32, tag="ubl_t")
            nc.sync.dma_start(out=ubl_t[:], in_=ubl[:])

            # HAM warm-up: a dependency-free back-to-back MM burst fires the
            # PE activity monitor (K=8/8 -> 2.4 GHz) before the pipeline
            # starts; the steady pipeline never idles a full MID window, so
            # the PE stays warm. Without this every MM runs at the cold
            # 1.2 GHz rate (observed on the previous kernel).
            # (k=128 full-array MMs: low-k MMs don't trip the activity
            # monitor — a k=28 warm-up burst was observed to never fire.)
            srw = consts.tile([128, B], dt.float16, tag="srw")
            nc.vector.memset(srw[:], 0.0)
            # All fillers write rotating 128-col quarters of ONE psum tile:
            # same-quarter deps are 4 MMs apart, so the stream is truly
            # back-to-back (a 1-deep write-write chain leaves ~8% idle gaps,
            # which was observed to keep the activity monitor from firing).
            wufill = pfill.tile([128, B], dt.float32, tag="wf", name="wufill")
            nfill = 0

            def emit_fill(n):
                nonlocal nfill
                for _ in range(n):
                    q = (nfill % 4) * 128
                    nfill += 1
                    nc.tensor.matmul(wufill[:, q : q + 128],
                                     lhsT=srw[:, 0:128], rhs=srw[:, 0:128],
                                     start=True, stop=True)

            # warm-up: ~60 cold N=128 MMs (107ns) fire the monitor at ~6µs,
            # the rest run warm (53ns) — bridges the NRT preamble + first
            # input-DMA latency (~14µs) with a dense full-array stream.
            emit_fill(150)

            xts = {}   # group -> xt tile
            ots = {}   # group -> output accumulation tile
            encs = {}  # supertile -> [ehi, elo, zhi, zlo]
            h1ss = {}  # supertile -> h1s tile
            h2ss = {}  # supertile -> h2s tile

            def ensure_group(g):
                if g in xts or g >= n_super // G:
                    return
                c0 = g * GROUP_COLS
                xt = xpool.tile([64, GROUP_COLS], dt.float16, tag="xt",
                                name=f"xt{g}")
                nc.sync.dma_start(out=xt[:], in_=xp[:, c0 : c0 + GROUP_COLS])
                xts[g] = xt
                ots[g] = outp.tile([128, GROUP_COLS], dt.float16, tag="ot",
                                   name=f"ot{g}")

            def emit_expand(t):
                g, j = divmod(t, G)
                xt = xts[g]
                cols = slice(j * B, (j + 1) * B)
                zhi = pzhi.tile([128, SUPER], dt.float32, tag="zhi",
                                name=f"zhi{t}")
                zlo = pzlo.tile([ENC - 128, SUPER], dt.float32, tag="zlo",
                                name=f"zlo{t}")
                ehi = encp.tile([128, SUPER], dt.float16, tag="ehi",
                                name=f"ehi{t}")
                elo = encp.tile([ENC - 128, SUPER], dt.float16, tag="elo",
                                name=f"elo{t}")
                encs[t] = [ehi, elo, zhi, zlo]
                # group 1: hi(block0) || lo(block1) — disjoint row strips
                nc.tensor.matmul(zhi[:, 0:B], lhsT=lwt[0:KR, 0:128],
                                 rhs=xt[0:KR, cols],
                                 start=True, stop=True, tile_position=(0, 0))
                nc.tensor.matmul(zlo[:, B : 2 * B],
                                 lhsT=lwt[32 : 32 + KR, 128:ENC],
                                 rhs=xt[32 : 32 + KR, cols],
                                 start=True, stop=True, tile_position=(32, 0))
                # group 2: lo(block0) || hi(block1) — emitted in this order so
                # each is pc-adjacent to its partner and only conflicts with
                # group-1 cells (lo0 vs hi0 rows 0:28; hi1 vs lo1 rows 32:60)
                nc.tensor.matmul(zlo[:, 0:B], lhsT=lwt[0:KR, 128:ENC],
                                 rhs=xt[0:KR, cols],
                                 start=True, stop=True, tile_position=(0, 0))
                nc.tensor.matmul(zhi[:, B : 2 * B],
                                 lhsT=lwt[32 : 32 + KR, 0:128],
                                 rhs=xt[32 : 32 + KR, cols],
                                 start=True, stop=True, tile_position=(32, 0))

            def emit_exp(t):
                ehi, elo, zhi, zlo = encs[t]
                nc.scalar.activation(ehi[:], zhi[:], Act.Exp, bias=ubh_t[:])
                nc.scalar.activation(elo[:], zlo[:], Act.Exp, bias=ubl_t[:])

            def emit_l1(t):
                ehi, elo = encs[t][0], encs[t][1]
                h1 = ph.tile([128, B], dt.float32, tag="hh", name=f"h1_{t}")
                nc.tensor.matmul(h1[0:64, :], lhsT=w1a_t[:], rhs=ehi[:, 0:B],
                                 start=True, stop=False, tile_position=(0, 0))
                nc.tensor.matmul(h1[64:128, :], lhsT=w1a_t[:],
                                 rhs=ehi[:, B : 2 * B],
                                 start=True, stop=False, tile_position=(0, 64))
                nc.tensor.matmul(h1[0:64, :], lhsT=w1b_t[:], rhs=elo[:, 0:B],
                                 start=False, stop=True, tile_position=(0, 0))
                nc.tensor.matmul(h1[64:128, :], lhsT=w1b_t[:],
                                 rhs=elo[:, B : 2 * B],
                                 start=False, stop=True, tile_position=(0, 64))
                del encs[t]
                h1s = hp.tile([128, B], dt.float16, tag="h1s", name=f"h1s{t}")
                nc.vector.tensor_scalar(h1s[:], h1[:], b1s_t[:], 0.0,
                                        Alu.add, Alu.max)
                h1ss[t] = h1s

            def emit_l2(t):
                h1s = h1ss.pop(t)
                h2 = ph.tile([128, B], dt.float32, tag="hh", name=f"h2_{t}")
                nc.tensor.matmul(h2[0:64, :], lhsT=w2s_t[0:64, :],
                                 rhs=h1s[0:64, :],
                                 start=True, stop=True, tile_position=(0, 0))
                nc.tensor.matmul(h2[64:128, :], lhsT=w2s_t[64:128, :],
                                 rhs=h1s[64:128, :],
                                 start=True, stop=True, tile_position=(64, 64))
                h2s = hp.tile([128, B], dt.float16, tag="h2s", name=f"h2s{t}")
                nc.vector.tensor_scalar(h2s[:], h2[:], b2s_t[:], 0.0,
                                        Alu.add, Alu.max)
                h2ss[t] = h2s

            def emit_l3(t):
                g, j = divmod(t, G)
                h2s = h2ss.pop(t)
                # Flipped quadrants: L3 occupies (0,64)/(64,0) so it pairs
                # with L2 of a later supertile on (0,0)/(64,64). Output rows
                # are therefore [block1; block0].
                op = pop.tile([128, B], dt.float32, tag="op", name=f"op{t}")
                nc.tensor.matmul(op[64:128, :], lhsT=w3s_t[0:64, :],
                                 rhs=h2s[0:64, :],
                                 start=True, stop=True, tile_position=(0, 64))
                nc.tensor.matmul(op[0:64, :], lhsT=w3s_t[64:128, :],
                                 rhs=h2s[64:128, :],
                                 start=True, stop=True, tile_position=(64, 0))
                nc.vector.tensor_scalar_add(ots[g][:, j * B : (j + 1) * B],
                                            op[:], b3s_t[:])
                if j == G - 1:
                    c0 = g * GROUP_COLS
                    nc.sync.dma_start(out=yt[:, c0 : c0 + GROUP_COLS],
                                      in_=ots[g][:])
                    del xts[g], ots[g]

            # PE keep-warm fillers: the activity monitor re-throttles the PE
            # clock to 1.2 GHz whenever a ~3.4µs window has >~20% idle — and
            # the ACT-bound pipeline inherently leaves the PE ~25% idle.
            # Dependency-free N=128 filler MMs (53ns warm) keep the stream
            # dense so every real MM runs at the warm 2.4 GHz rate (real MM
            # slots then cost ~220ns instead of ~470ns).
            ramp = {0: 20, 1: 14, 2: 8, 3: 8, 4: 6, 5: 6, 6: 4, 7: 4}

            # Pipeline: expand/exp(t) | L1/L2(t-2) | L3(t-3)
            for t in range(n_super + 3):
                if t < n_super:
                    ensure_group(t // G)
                    emit_expand(t)
                    emit_exp(t)
                    emit_fill(4 + ramp.get(t, 0))
                if 0 <= t - 2 < n_super:
                    emit_l1(t - 2)
                    emit_l2(t - 2)
                    emit_fill(4)
                if 0 <= t - 3 < n_super:
                    emit_l3(t - 3)

    nc.finalize()
    return nc


def _get_nc():
    key = N_SUPER
    if key not in _BUILD_CACHE:
        _BUILD_CACHE[key] = _build_bass(key)
    return _BUILD_CACHE[key]


def _f16_hilo(x64):
    """Exact hi/lo split: x ~= hi + lo with hi, lo fp16 (inputs are fp64)."""
    hi = x64.astype(np.float16)
    lo = (x64 - hi.astype(np.float64)).astype(np.float16)
    return hi, lo


def _pack_weights(W1, b1, W2, b2, W3, b3):
    j = np.arange(BINS, dtype=np.float64)
    jrow = np.tile(j, IN_DIMS)  # [224]: bin index per enc column
    L = np.zeros((KR, ENC), np.float16)
    for d in range(IN_DIMS):
        cols = slice(d * BINS, (d + 1) * BINS)
        L[d, cols] = j.astype(np.float16)       # s_hi
        L[7 + d, cols] = j.astype(np.float16)   # s_lo
        L[14 + d, cols] = np.float16(-512.0)    # q_hi
        L[21 + d, cols] = np.float16(-512.0)    # q_lo
    u = (-512.0 * jrow * jrow / 961.0).astype(np.float32)  # [224]

    w1 = W1.astype(np.float16)
    return {
        "lw": L,
        "w1a": np.ascontiguousarray(w1[0:128]),
        "w1b": np.ascontiguousarray(w1[128:ENC]),
        "w2s": np.concatenate([W2, W2], 0).astype(np.float16),
        "w3s": np.concatenate([W3, W3], 0).astype(np.float16),
        "b1s": np.concatenate([b1, b1], 0).astype(np.float32).reshape(128, 1),
        "b2s": np.concatenate([b2, b2], 0).astype(np.float32).reshape(128, 1),
        "b3s": np.concatenate([b3, b3], 0).astype(np.float32).reshape(128, 1),
        "ubh": np.ascontiguousarray(u[0:128]).reshape(128, 1),
        "ubl": np.ascontiguousarray(u[128:ENC]).reshape(ENC - 128, 1),
    }


def _pack_inputs(pos01, wi01, rough01):
    x = np.concatenate(
        [np.asarray(pos01), np.asarray(wi01), np.asarray(rough01)], axis=1
    ).astype(np.float32)
    np.clip(x, 0.0, 1.0, out=x)
    x64 = x.astype(np.float64)
    s64 = x64 * S_SCALE
    q64 = x64 * x64
    s_hi, s_lo = _f16_hilo(s64)
    q_hi, q_lo = _f16_hilo(q64)
    P = np.concatenate([s_hi, s_lo, q_hi, q_lo], axis=1)  # [N, 28] fp16
    return np.ascontiguousarray(P.T)  # [28, N]


def kernel(pos01, wi01, rough01, W1, b1, W2, b2, W3, b3, centers):
    global LAST_RESULTS
    import os

    from concourse.bass_utils import run_bass_kernel_spmd

    nc = _get_nc()

    Pt = _pack_inputs(pos01, wi01, rough01)  # [28, N_TOTAL]
    wpacks = _pack_weights(
        np.asarray(W1), np.asarray(b1), np.asarray(W2), np.asarray(b2),
        np.asarray(W3), np.asarray(b3),
    )

    in_maps = []
    for c in range(N_CORES):
        m = dict(wpacks)
        r0 = c * NC_RAYS
        xp_c = np.zeros((64, HALF), np.float16)
        xp_c[0:KR] = Pt[:, r0 : r0 + HALF]
        xp_c[32 : 32 + KR] = Pt[:, r0 + HALF : r0 + NC_RAYS]
        m["xp"] = xp_c
        in_maps.append(m)

    trace = bool(int(os.environ.get("KERNEL_TRACE", "0")))
    res = run_bass_kernel_spmd(nc, in_maps, list(range(N_CORES)), trace=trace)
    LAST_RESULTS = res

    out = np.empty((N_TOTAL, OUT), np.float32)
    for c in range(N_CORES):
        ytc = res.results[c]["yt"]  # [128, HALF] fp16
        r0 = c * NC_RAYS
        # flipped L3 quadrants: block0 rays on rows 64:128, block1 on 0:64
        out[r0 : r0 + HALF] = ytc[64:128].T.astype(np.float32)
        out[r0 + HALF : r0 + NC_RAYS] = ytc[0:64].T.astype(np.float32)
    return out


# revision 16
# speedup vs baseline: 1.3886x; 1.0454x over previous
"""Trainium2 Bass kernel for a OneBlob-encoded 3-layer MLP (ConditioningNetwork).

Math:  x = clip(concat(pos01, wi01, rough01), 0, 1)          [N, 7]
       enc[n, d*32+j] = exp(-0.5 ((x[n,d]-c[j]) / sigma)^2)  [N, 224], sigma = 1/32
       y = relu(relu(enc@W1+b1)@W2+b2)@W3+b3                 [N, 64]

Strategy (pure data parallel over 8 cores, weights replicated):
  - z[dj] = -512 x_d^2 + j*(1024/31) x_d - 512 j^2/961 is affine in
    (s, q) = ((1024/31) x, x^2) with EXACT fp16 weights (integers j and -512);
    s and q are fed as exact fp16 hi+lo pairs from the host (28 rows/ray).
    The constant term -512 j^2/961 rides in the Exp activation's per-partition
    bias (fp32), so no "ones" rows are needed.
  - Rays are split into two blocks (first/second half of the core's rays)
    packed at SBUF partitions 0:28 and 32:60 -> expand matmuls for the two
    blocks run CONCURRENTLY on disjoint PE row strips (no input duplication;
    input DMA is 64 B/ray, 4x less than a 128-row layout).
  - enc = Exp(z + u) on ACT straight out of PSUM, fp16 into SBUF.
  - 3 MLP matmuls in fp16 (fp32 PSUM): per supertile (512 rays/block x 2)
    the PE issues 5 concurrency groups:
      [hi0||lo1] [hi1||lo0] [w1aA||w1aB] [w1bA||w1bB] [L2A||L2B||L3A'||L3B']
    (L3 uses flipped quadrants so it pairs with L2 of the next supertile.)
  - Bias+ReLU are single DVE tensor_scalar ops over packed [128, 512] PSUM
    tiles; output is written fp16 (halves output DMA), unpacked on the host.

Input row packing per block (fp16, 28 rows): 0:7 s_hi, 7:14 s_lo,
14:21 q_hi, 21:28 q_lo.  Output yt [128, nc/2] fp16: rows 64:128 = block0
rays (flipped L3 quadrants), rows 0:64 = block1 rays; column = ray index
within block.
"""

import sys

import numpy as np

if "/opt/trn_rl_repo" not in sys.path:
    sys.path.insert(0, "/opt/trn_rl_repo")

N_CORES = 8
N_TOTAL = 1048576
NC_RAYS = N_TOTAL // N_CORES  # 131072 rays per core
HALF = NC_RAYS // 2  # 65536 rays per block
BINS = 32
HID = 64
OUT = 64
IN_DIMS = 7
ENC = IN_DIMS * BINS  # 224

KR = 28  # packed rows per block (s_hi, s_lo, q_hi, q_lo)
B = 512  # rays per block per supertile (one fp32 PSUM bank)
SUPER = 2 * B  # rays per supertile (block0 + block1)
G = 8  # supertiles per DMA group
GROUP_COLS = G * B  # 4096 cols per block per group
N_SUPER = NC_RAYS // SUPER  # 128
N_GROUPS = N_SUPER // G  # 16

S_SCALE = 1024.0 / 31.0

# Set by the last kernel() call so a test harness can read profile/exec time.
LAST_RESULTS = None

_BUILD_CACHE = {}


def _build_bass(n_super):
    import concourse.tile as tile
    from concourse import bacc, mybir

    dt = mybir.dt
    Act = mybir.ActivationFunctionType
    Alu = mybir.AluOpType

    nc = bacc.Bacc("TRN2", target_bir_lowering=False, debug=False)

    xp = nc.dram_tensor("xp", [64, HALF], dt.float16, kind="ExternalInput")
    lw = nc.dram_tensor("lw", [KR, ENC], dt.float16, kind="ExternalInput")
    w1a = nc.dram_tensor("w1a", [128, HID], dt.float16, kind="ExternalInput")
    w1b = nc.dram_tensor("w1b", [ENC - 128, HID], dt.float16, kind="ExternalInput")
    w2s = nc.dram_tensor("w2s", [128, HID], dt.float16, kind="ExternalInput")
    w3s = nc.dram_tensor("w3s", [128, OUT], dt.float16, kind="ExternalInput")
    b1s = nc.dram_tensor("b1s", [128, 1], dt.float32, kind="ExternalInput")
    b2s = nc.dram_tensor("b2s", [128, 1], dt.float32, kind="ExternalInput")
    b3s = nc.dram_tensor("b3s", [128, 1], dt.float32, kind="ExternalInput")
    ubh = nc.dram_tensor("ubh", [128, 1], dt.float32, kind="ExternalInput")
    ubl = nc.dram_tensor("ubl", [ENC - 128, 1], dt.float32, kind="ExternalInput")
    yt = nc.dram_tensor("yt", [128, HALF], dt.float16, kind="ExternalOutput")

    with tile.TileContext(nc) as tc:
        with (
            tc.tile_pool(name="consts", bufs=1) as consts,
            tc.tile_pool(name="xpool", bufs=3) as xpool,
            tc.tile_pool(name="encp", bufs=6) as encp,
            tc.tile_pool(name="hp", bufs=4) as hp,
            tc.tile_pool(name="outp", bufs=2) as outp,
            tc.tile_pool(name="pzhi", bufs=1, space="PSUM") as pzhi,
            tc.tile_pool(name="pzlo", bufs=1, space="PSUM") as pzlo,
            tc.tile_pool(name="ph", bufs=2, space="PSUM") as ph,
            tc.tile_pool(name="pop", bufs=1, space="PSUM") as pop,
            tc.tile_pool(name="pfill", bufs=1, space="PSUM") as pfill,
        ):
            # expand weights, replicated at partitions 0:28 (block0) and
            # 32:60 (block1) to match the PE row strips of each block's rhs
            lwt = consts.tile([32 + KR, ENC], dt.float16, tag="lwt")
            nc.sync.dma_start(out=lwt[0:KR, :], in_=lw[:])
            nc.sync.dma_start(out=lwt[32 : 32 + KR, :], in_=lw[:])
            w1a_t = consts.tile([128, HID], dt.float16, tag="w1a_t")
            nc.sync.dma_start(out=w1a_t[:], in_=w1a[:])
            w1b_t = consts.tile([ENC - 128, HID], dt.float16, tag="w1b_t")
            nc.sync.dma_start(out=w1b_t[:], in_=w1b[:])
            w2s_t = consts.tile([128, HID], dt.float16, tag="w2s_t")
            nc.sync.dma_start(out=w2s_t[:], in_=w2s[:])
            w3s_t = consts.tile([128, OUT], dt.float16, tag="w3s_t")
            nc.sync.dma_start(out=w3s_t[:], in_=w3s[:])
            b1s_t = consts.tile([128, 1], dt.float32, tag="b1s_t")
            nc.sync.dma_start(out=b1s_t[:], in_=b1s[:])
            b2s_t = consts.tile([128, 1], dt.float32, tag="b2s_t")
            nc.sync.dma_start(out=b2s_t[:], in_=b2s[:])
            b3s_t = consts.tile([128, 1], dt.float32, tag="b3s_t")
            nc.sync.dma_start(out=b3s_t[:], in_=b3s[:])
            ubh_t = consts.tile([128, 1], dt.float32, tag="ubh_t")
            nc.sync.dma_start(out=ubh_t[:], in_=ubh[:])
            ubl_t = consts.tile([ENC - 128, 1], dt.float32, tag="ubl_t")
            nc.sync.dma_start(out=ubl_t[:], in_=ubl[:])

            # HAM warm-up: a dependency-free back-to-back MM burst fires the
            # PE activity monitor (K=8/8 -> 2.4 GHz) before the pipeline
            # starts; the steady pipeline never idles a full MID window, so
            # the PE stays warm. Without this every MM runs at the cold
            # 1.2 GHz rate (observed on the previous kernel).
            # (k=128 full-array MMs: low-k MMs don't trip the activity
            # monitor — a k=28 warm-up burst was observed to never fire.)
            srw = consts.tile([128, B], dt.float16, tag="srw")
            nc.vector.memset(srw[:], 0.0)
            # All fillers write rotating 128-col quarters of ONE psum tile:
            # same-quarter deps are 4 MMs apart, so the stream is truly
            # back-to-back (a 1-deep write-write chain leaves ~8% idle gaps,
            # which was observed to keep the activity monitor from firing).
            wufill = pfill.tile([128, B], dt.float32, tag="wf", name="wufill")
            nfill = 0

            def emit_fill(n):
                nonlocal nfill
                for _ in range(n):
                    q = (nfill % 4) * 128
                    nfill += 1
                    nc.tensor.matmul(wufill[:, q : q + 128],
                                     lhsT=srw[:, 0:128], rhs=srw[:, 0:128],
                                     start=True, stop=True)

            # warm-up: ~60 cold N=128 MMs (107ns) fire the monitor at ~6µs,
            # the rest run warm (53ns) — bridges the NRT preamble + first
            # input-DMA latency (~14µs) with a dense full-array stream.
            emit_fill(150)

            xts = {}   # group -> xt tile
            ots = {}   # group -> output accumulation tile
            encs = {}  # supertile -> [ehi, elo, zhi, zlo]
            h1ss = {}  # supertile -> h1s tile
            h2ss = {}  # supertile -> h2s tile

            def ensure_group(g):
                if g in xts or g >= n_super // G:
                    return
                c0 = g * GROUP_COLS
                xt = xpool.tile([64, GROUP_COLS], dt.float16, tag="xt",
                                name=f"xt{g}")
                nc.sync.dma_start(out=xt[:], in_=xp[:, c0 : c0 + GROUP_COLS])
                xts[g] = xt
                ots[g] = outp.tile([128, GROUP_COLS], dt.float16, tag="ot",
                                   name=f"ot{g}")

            def emit_expand(t):
                g, j = divmod(t, G)
                xt = xts[g]
                cols = slice(j * B, (j + 1) * B)
                zhi = pzhi.tile([128, SUPER], dt.float32, tag="zhi",
                                name=f"zhi{t}")
                zlo = pzlo.tile([ENC - 128, SUPER], dt.float32, tag="zlo",
                                name=f"zlo{t}")
                ehi = encp.tile([128, SUPER], dt.float16, tag="ehi",
                                name=f"ehi{t}")
                elo = encp.tile([ENC - 128, SUPER], dt.float16, tag="elo",
                                name=f"elo{t}")
                encs[t] = [ehi, elo, zhi, zlo]
                # group 1: hi(block0) || hi(block1) — both zhi writers first,
                # so exp-hi(t) only waits on this pair; the pair itself only
                # needs zhi free (exp-hi(t-1) done) and runs during exp-lo.
                nc.tensor.matmul(zhi[:, 0:B], lhsT=lwt[0:KR, 0:128],
                                 rhs=xt[0:KR, cols],
                                 start=True, stop=True, tile_position=(0, 0))
                nc.tensor.matmul(zhi[:, B : 2 * B],
                                 lhsT=lwt[32 : 32 + KR, 0:128],
                                 rhs=xt[32 : 32 + KR, cols],
                                 start=True, stop=True, tile_position=(32, 0))
                # group 2: lo(block0) || lo(block1) — runs during exp-hi(t)
                nc.tensor.matmul(zlo[:, 0:B], lhsT=lwt[0:KR, 128:ENC],
                                 rhs=xt[0:KR, cols],
                                 start=True, stop=True, tile_position=(0, 0))
                nc.tensor.matmul(zlo[:, B : 2 * B],
                                 lhsT=lwt[32 : 32 + KR, 128:ENC],
                                 rhs=xt[32 : 32 + KR, cols],
                                 start=True, stop=True, tile_position=(32, 0))

            def emit_exp(t):
                ehi, elo, zhi, zlo = encs[t]
                nc.scalar.activation(ehi[:], zhi[:], Act.Exp, bias=ubh_t[:])
                nc.scalar.activation(elo[:], zlo[:], Act.Exp, bias=ubl_t[:])

            def emit_l1(t):
                ehi, elo = encs[t][0], encs[t][1]
                h1 = ph.tile([128, B], dt.float32, tag="hh", name=f"h1_{t}")
                nc.tensor.matmul(h1[0:64, :], lhsT=w1a_t[:], rhs=ehi[:, 0:B],
                                 start=True, stop=False, tile_position=(0, 0))
                nc.tensor.matmul(h1[64:128, :], lhsT=w1a_t[:],
                                 rhs=ehi[:, B : 2 * B],
                                 start=True, stop=False, tile_position=(0, 64))
                nc.tensor.matmul(h1[0:64, :], lhsT=w1b_t[:], rhs=elo[:, 0:B],
                                 start=False, stop=True, tile_position=(0, 0))
                nc.tensor.matmul(h1[64:128, :], lhsT=w1b_t[:],
                                 rhs=elo[:, B : 2 * B],
                                 start=False, stop=True, tile_position=(0, 64))
                del encs[t]
                h1s = hp.tile([128, B], dt.float16, tag="h1s", name=f"h1s{t}")
                nc.vector.tensor_scalar(h1s[:], h1[:], b1s_t[:], 0.0,
                                        Alu.add, Alu.max)
                h1ss[t] = h1s

            def emit_l2(t):
                h1s = h1ss.pop(t)
                h2 = ph.tile([128, B], dt.float32, tag="hh", name=f"h2_{t}")
                nc.tensor.matmul(h2[0:64, :], lhsT=w2s_t[0:64, :],
                                 rhs=h1s[0:64, :],
                                 start=True, stop=True, tile_position=(0, 0))
                nc.tensor.matmul(h2[64:128, :], lhsT=w2s_t[64:128, :],
                                 rhs=h1s[64:128, :],
                                 start=True, stop=True, tile_position=(64, 64))
                h2s = hp.tile([128, B], dt.float16, tag="h2s", name=f"h2s{t}")
                nc.vector.tensor_scalar(h2s[:], h2[:], b2s_t[:], 0.0,
                                        Alu.add, Alu.max)
                h2ss[t] = h2s

            def emit_l3(t):
                g, j = divmod(t, G)
                h2s = h2ss.pop(t)
                # Flipped quadrants: L3 occupies (0,64)/(64,0) so it pairs
                # with L2 of a later supertile on (0,0)/(64,64). Output rows
                # are therefore [block1; block0].
                op = pop.tile([128, B], dt.float32, tag="op", name=f"op{t}")
                nc.tensor.matmul(op[64:128, :], lhsT=w3s_t[0:64, :],
                                 rhs=h2s[0:64, :],
                                 start=True, stop=True, tile_position=(0, 64))
                nc.tensor.matmul(op[0:64, :], lhsT=w3s_t[64:128, :],
                                 rhs=h2s[64:128, :],
                                 start=True, stop=True, tile_position=(64, 0))
                nc.vector.tensor_scalar_add(ots[g][:, j * B : (j + 1) * B],
                                            op[:], b3s_t[:])
                if j == G - 1:
                    c0 = g * GROUP_COLS
                    nc.sync.dma_start(out=yt[:, c0 : c0 + GROUP_COLS],
                                      in_=ots[g][:])
                    del xts[g], ots[g]

            # PE keep-warm fillers: the activity monitor re-throttles the PE
            # clock to 1.2 GHz whenever a ~3.4µs window has >~20% idle — and
            # the ACT-bound pipeline inherently leaves the PE ~25% idle.
            # Dependency-free N=128 filler MMs (53ns warm) keep the stream
            # dense so every real MM runs at the warm 2.4 GHz rate (real MM
            # slots then cost ~220ns instead of ~470ns).
            ramp = {0: 20, 1: 14, 2: 8, 3: 8, 4: 6, 5: 6, 6: 4, 7: 4}

            # Pipeline: expand/exp(t) | L1/L2(t-2) | L3(t-3)
            for t in range(n_super + 3):
                if t < n_super:
                    ensure_group(t // G)
                    emit_expand(t)
                    emit_exp(t)
                    emit_fill(4 + ramp.get(t, 0))
                if 0 <= t - 2 < n_super:
                    emit_l1(t - 2)
                    emit_l2(t - 2)
                    emit_fill(4)
                if 0 <= t - 3 < n_super:
                    emit_l3(t - 3)

    nc.finalize()
    return nc


def _get_nc():
    key = N_SUPER
    if key not in _BUILD_CACHE:
        _BUILD_CACHE[key] = _build_bass(key)
    return _BUILD_CACHE[key]


def _f16_hilo(x64):
    """Exact hi/lo split: x ~= hi + lo with hi, lo fp16 (inputs are fp64)."""
    hi = x64.astype(np.float16)
    lo = (x64 - hi.astype(np.float64)).astype(np.float16)
    return hi, lo


def _pack_weights(W1, b1, W2, b2, W3, b3):
    j = np.arange(BINS, dtype=np.float64)
    jrow = np.tile(j, IN_DIMS)  # [224]: bin index per enc column
    L = np.zeros((KR, ENC), np.float16)
    for d in range(IN_DIMS):
        cols = slice(d * BINS, (d + 1) * BINS)
        L[d, cols] = j.astype(np.float16)       # s_hi
        L[7 + d, cols] = j.astype(np.float16)   # s_lo
        L[14 + d, cols] = np.float16(-512.0)    # q_hi
        L[21 + d, cols] = np.float16(-512.0)    # q_lo
    u = (-512.0 * jrow * jrow / 961.0).astype(np.float32)  # [224]

    w1 = W1.astype(np.float16)
    return {
        "lw": L,
        "w1a": np.ascontiguousarray(w1[0:128]),
        "w1b": np.ascontiguousarray(w1[128:ENC]),
        "w2s": np.concatenate([W2, W2], 0).astype(np.float16),
        "w3s": np.concatenate([W3, W3], 0).astype(np.float16),
        "b1s": np.concatenate([b1, b1], 0).astype(np.float32).reshape(128, 1),
        "b2s": np.concatenate([b2, b2], 0).astype(np.float32).reshape(128, 1),
        "b3s": np.concatenate([b3, b3], 0).astype(np.float32).reshape(128, 1),
        "ubh": np.ascontiguousarray(u[0:128]).reshape(128, 1),
        "ubl": np.ascontiguousarray(u[128:ENC]).reshape(ENC - 128, 1),
    }


def _pack_inputs(pos01, wi01, rough01):
    x = np.concatenate(
        [np.asarray(pos01), np.asarray(wi01), np.asarray(rough01)], axis=1
    ).astype(np.float32)
    np.clip(x, 0.0, 1.0, out=x)
    x64 = x.astype(np.float64)
    s64 = x64 * S_SCALE
    q64 = x64 * x64
    s_hi, s_lo = _f16_hilo(s64)
    q_hi, q_lo = _f16_hilo(q64)
    P = np.concatenate([s_hi, s_lo, q_hi, q_lo], axis=1)  # [N, 28] fp16
    return np.ascontiguousarray(P.T)  # [28, N]


def kernel(pos01, wi01, rough01, W1, b1, W2, b2, W3, b3, centers):
    global LAST_RESULTS
    import os

    from concourse.bass_utils import run_bass_kernel_spmd

    nc = _get_nc()

    Pt = _pack_inputs(pos01, wi01, rough01)  # [28, N_TOTAL]
    wpacks = _pack_weights(
        np.asarray(W1), np.asarray(b1), np.asarray(W2), np.asarray(b2),
        np.asarray(W3), np.asarray(b3),
    )

    in_maps = []
    for c in range(N_CORES):
        m = dict(wpacks)
        r0 = c * NC_RAYS
        xp_c = np.zeros((64, HALF), np.float16)
        xp_c[0:KR] = Pt[:, r0 : r0 + HALF]
        xp_c[32 : 32 + KR] = Pt[:, r0 + HALF : r0 + NC_RAYS]
        m["xp"] = xp_c
        in_maps.append(m)

    trace = bool(int(os.environ.get("KERNEL_TRACE", "0")))
    res = run_bass_kernel_spmd(nc, in_maps, list(range(N_CORES)), trace=trace)
    LAST_RESULTS = res

    out = np.empty((N_TOTAL, OUT), np.float32)
    for c in range(N_CORES):
        ytc = res.results[c]["yt"]  # [128, HALF] fp16
        r0 = c * NC_RAYS
        # flipped L3 quadrants: block0 rays on rows 64:128, block1 on 0:64
        out[r0 : r0 + HALF] = ytc[64:128].T.astype(np.float32)
        out[r0 + HALF : r0 + NC_RAYS] = ytc[0:64].T.astype(np.float32)
    return out
